# revision 40
# baseline (speedup 1.0000x reference)
"""Trainium2 Bass kernel for nn_AttentionLayer (B=2, S=2048, HID=1024, 16 heads x 64).

Sharding: 8 cores = 2 batches x 4 head-groups (4 heads each). Each core computes
its batch's attention for its 4 heads and writes a disjoint [256, 2048] slice of
the output (transposed). No collectives.

Device math (per core, all layouts feature-major to match TensorE):
  Q.T = Wq_g.T @ X_f.T + bq_g      [256 j, 2048 f]   (f32r matmuls)
  K.T = Wk_g.T @ X_t.T + bk_g      [256 j, 2048 t]
  V'  = [X_t.T.T @ Wv_g + 1*bv_g | ones]  [2048 t, 4, 65] (bf16)
  S.T = K.T_h.T @ Q.T_h            [t, f] per head    (row-packed pairs, K=64)
  E   = exp(S.T / 8)               (bf16, ScalarE)
  ctx'.T = V'_h.T @ E              [65, f]; row 64 = softmax denominator
  out = ctx'.T[0:64] * (1/denom broadcast)            (already includes bv)

Phase B is a software pipeline over units (tt, dd): ST (2 matmuls) -> exp
(1 ScalarE activation [128,1024]) -> PV (2 accumulating matmuls), with PSUM
split as 2 double-buffered ST slots (2 banks each) + 4 chain accumulators
(1 bank each). This keeps the PE free of >3.4us idle windows (HAM stays at
8/8) and keeps the ScalarE exp stream (the throughput floor) saturated.
"""
import numpy as np

B, S, HID = 2, 2048, 1024
NUM_HEADS, HEAD_DIM = 16, 64
G = 4                 # head-groups (cores per batch)
HPC = 4               # heads per core
JW = HPC * HEAD_DIM   # 256 W columns per core
NC_CHUNKS = HID // 128  # 8 contraction chunks
NT = S // 128         # 16 t tiles
NFB = 2               # f blocks of 1024
SCALE = 1.0 / np.sqrt(float(HEAD_DIM))

_cached = None


def _build():
    import contextlib
    import concourse.bass as bass
    import concourse.tile as tile
    from concourse import bacc, mybir

    F32R = mybir.dt.float32r
    F32 = mybir.dt.float32
    BF16 = mybir.dt.bfloat16
    Act = mybir.ActivationFunctionType

    nc = bacc.Bacc("TRN2", target_bir_lowering=False, debug=False, num_devices=8)

    fromT = nc.dram_tensor("fromT", (HID, S), BF16, kind="ExternalInput").ap()
    toT = nc.dram_tensor("toT", (HID, S), BF16, kind="ExternalInput").ap()
    wq = nc.dram_tensor("wq", (HID, JW), BF16, kind="ExternalInput").ap()
    wk = nc.dram_tensor("wk", (HID, JW), BF16, kind="ExternalInput").ap()
    wv = nc.dram_tensor("wv", (HID, JW), BF16, kind="ExternalInput").ap()
    bq = nc.dram_tensor("bq", (JW, 1), F32, kind="ExternalInput").ap()
    bk = nc.dram_tensor("bk", (JW, 1), F32, kind="ExternalInput").ap()
    bv = nc.dram_tensor("bv", (1, JW), F32R, kind="ExternalInput").ap()
    out = nc.dram_tensor("out", (JW, S), F32, kind="ExternalOutput").ap()

    with tile.TileContext(nc) as tc:
        with contextlib.ExitStack() as es:
            persist = es.enter_context(tc.tile_pool(name="persist", bufs=1))

            # --- constants / biases
            bq_sb = []
            bk_sb = []
            for jt in range(2):
                tq = persist.tile([128, 1], F32, tag=f"bq{jt}")
                nc.sync.dma_start(tq[:], bq[128 * jt:128 * jt + 128, 0:1])
                bq_sb.append(tq)
                tk = persist.tile([128, 1], F32, tag=f"bk{jt}")
                nc.sync.dma_start(tk[:], bk[128 * jt:128 * jt + 128, 0:1])
                bk_sb.append(tk)
            bv_row = persist.tile([1, JW], F32R, tag="bvrow")
            nc.sync.dma_start(bv_row[:], bv[0:1, :])
            ones_f = persist.tile([1, 128], F32, tag="onesf")
            nc.vector.memset(ones_f[:], 1.0)
            ones_r = persist.tile([1, 128], F32R, tag="onesr")
            nc.vector.tensor_copy(ones_r[:], ones_f[:])
            ones_bf = persist.tile([1, 64], BF16, tag="onesbf")
            nc.vector.memset(ones_bf[:], 1.0)

            # --- persistent projection outputs
            qt = [persist.tile([128, S], F32R, tag=f"qt{jt}", name=f"qt{jt}") for jt in range(2)]
            kt = [persist.tile([128, S], F32R, tag=f"kt{jt}", name=f"kt{jt}") for jt in range(2)]
            vp = [persist.tile([128, HPC, 65], BF16, tag=f"vp{tt}", name=f"vp{tt}") for tt in range(NT)]

            # Phase-B SBUF/PSUM pools are opened BEFORE the phase-A PSUM
            # pools (pool release is LIFO): stp(4 banks) coexists with
            # psA(2)+psAv(2) during phase A, then cap(4) opens once the
            # phase-A pools close. This lets round 0's ST+exp stream start
            # inside phase A, right after the K projections.
            epool = es.enter_context(tc.tile_pool(name="epool", bufs=18))
            spool = es.enter_context(tc.tile_pool(name="spool", bufs=4))
            stp = es.enter_context(tc.tile_pool(name="stp", bufs=2, space="PSUM"))

            FW = 512  # f-block width per round

            def emit_st_exp(jt, fb, tt):
                fo = FW * fb
                st = stp.tile([128, 1024], mybir.dt.float32,
                              tag="st", name=f"st{jt}_{fb}_{tt}")
                for dd in range(2):
                    nc.tensor.matmul(
                        st[:, 512 * dd:512 * dd + FW],
                        kt[jt][64 * dd:64 * dd + 64, 128 * tt:128 * tt + 128],
                        qt[jt][64 * dd:64 * dd + 64, fo:fo + FW],
                        start=True, stop=True,
                        tile_position=(64 * dd, 0))
                e = epool.tile([128, 1024], BF16, tag="E",
                               name=f"e{jt}_{fb}_{tt}")
                nc.scalar.activation(e[:], st[:], Act.Exp, bias=0.0, scale=SCALE)
                return e

            def emit_epilogue_tail(cacc, rcp, k_local, fo, rnd):
                # Broadcast 1/denom over 64 partitions via a tiny PE
                # matmul into a rotating ST slot, then scale + store.
                bt = stp.tile([128, 1024], mybir.dt.float32,
                              tag="st", name=f"bt{rnd}_{k_local}_{fo}")
                nc.tensor.matmul(bt[0:64, 0:FW], ones_bf[0:1, :], rcp[:],
                                 start=True, stop=True)
                bcs = spool.tile([64, FW], F32, tag="bcs")
                nc.vector.tensor_copy(bcs[:], bt[0:64, 0:FW])
                so = spool.tile([64, FW], F32, tag="so")
                nc.vector.tensor_mul(so[:], cacc[0:64, :], bcs[:])
                nc.sync.dma_start(
                    out[64 * k_local:64 * k_local + 64, fo:fo + FW],
                    so[:])

            def emit_recips(chains, jt, fo, rnd, pending, on_act=False):
                for dd in range(2):
                    cacc = chains[dd]
                    rcp = spool.tile([1, FW], BF16, tag="rcp")
                    if on_act:
                        # drain-tail path: 1/d = exp(-ln d) on the (idle)
                        # ScalarE -- ~2x faster than the serial DVE recips
                        lnt = spool.tile([1, FW], F32, tag="lnt")
                        nc.scalar.activation(lnt[:], cacc[64:65, :], Act.Ln,
                                             bias=0.0, scale=1.0)
                        nc.scalar.activation(rcp[:], lnt[:], Act.Exp,
                                             bias=0.0, scale=-1.0)
                    else:
                        with nc.allow_low_precision(reason="softmax recip; tol 2e-2"):
                            nc.vector.reciprocal(rcp[:], cacc[64:65, :])
                    pending.append((cacc, rcp, 2 * jt + dd, fo, rnd))

            pending = []

            # --- phase A: projections (+ round-0 ST/exp prologue)
            E0 = []
            with contextlib.ExitStack() as esA:
                pA = esA.enter_context(tc.tile_pool(name="phaseA", bufs=1))
                psA = esA.enter_context(tc.tile_pool(name="psA", bufs=2, space="PSUM"))
                psAv = esA.enter_context(tc.tile_pool(name="psAv", bufs=2, space="PSUM"))
                wq_sb = pA.tile([128, NC_CHUNKS, JW], BF16, tag="wq")
                nc.sync.dma_start(wq_sb[:], wq.rearrange("(c p) j -> p c j", p=128))
                wk_sb = pA.tile([128, NC_CHUNKS, JW], BF16, tag="wk")
                nc.gpsimd.dma_start(wk_sb[:], wk.rearrange("(c p) j -> p c j", p=128))
                wv_sb = pA.tile([128, NC_CHUNKS, JW], BF16, tag="wv")
                nc.scalar.dma_start(wv_sb[:], wv.rearrange("(c p) j -> p c j", p=128))

                # stream fromT/toT as full-row [128, S] bf16 chunk tiles
                # (4KB/partition, fully contiguous DRAM reads), DMAs spread
                # across the three DMA-capable engine queues.
                dma_engines = [nc.sync, nc.gpsimd, nc.scalar]

                def stream(src, nm):
                    tiles = []
                    for c in range(NC_CHUNKS):
                        t = pA.tile([128, S], BF16, tag="xc",
                                    name=f"x{nm}{c}", bufs=16)
                        dma_engines[c % 3].dma_start(
                            t[:], src[128 * c:128 * c + 128, :])
                        tiles.append(t)
                    return tiles

                fx = stream(fromT, "f")
                tx = stream(toT, "t")

                for jt in range(2):
                    for fc in range(4):
                        acc = psA.tile([128, 512], mybir.dt.float32, tag="big")
                        for c in range(NC_CHUNKS):
                            nc.tensor.matmul(
                                acc[:],
                                wq_sb[:, c, 128 * jt:128 * jt + 128],
                                fx[c][:, 512 * fc:512 * fc + 512],
                                start=(c == 0), stop=(c == NC_CHUNKS - 1))
                        nc.vector.tensor_scalar_add(
                            qt[jt][:, 512 * fc:512 * fc + 512], acc[:],
                            bq_sb[jt][:])
                # K projections, with the round-0 ST/exp prologue interleaved
                # per jt=0 column block: the exp stream (the throughput
                # floor) starts as soon as the first K chain lands instead of
                # after all eight.
                for jt in range(2):
                    for fc in range(4):
                        acc = psA.tile([128, 512], mybir.dt.float32, tag="big")
                        for c in range(NC_CHUNKS):
                            nc.tensor.matmul(
                                acc[:],
                                wk_sb[:, c, 128 * jt:128 * jt + 128],
                                tx[c][:, 512 * fc:512 * fc + 512],
                                start=(c == 0), stop=(c == NC_CHUNKS - 1))
                        nc.vector.tensor_scalar_add(
                            kt[jt][:, 512 * fc:512 * fc + 512], acc[:],
                            bk_sb[jt][:])
                        if jt == 0:
                            for tt in range(4 * fc, 4 * fc + 4):
                                E0.append(emit_st_exp(0, 0, tt))

                for tt in range(NT):
                    accv = psAv.tile([128, 512], mybir.dt.float32, tag="sm")
                    nc.tensor.matmul(accv[:, 0:JW], ones_r[0:1, :], bv_row[0:1, :],
                                     start=True, stop=False)
                    for c in range(NC_CHUNKS):
                        nc.tensor.matmul(
                            accv[:, 0:JW],
                            tx[c][:, 128 * tt:128 * tt + 128],
                            wv_sb[:, c, :],
                            start=False, stop=(c == NC_CHUNKS - 1))
                    nc.vector.memset(vp[tt][:, :, 64], 1.0)
                    nc.vector.tensor_copy(
                        vp[tt][:, :, 0:64],
                        accv[:, 0:JW].rearrange("p (k e) -> p k e", k=HPC))

            # --- phase B: attention, software-pipelined per tt unit.
            # Rounds are (jt, fb) with 512-wide f-blocks so each round only
            # holds 2 chain accumulators (1 PSUM bank each) -> chains double-
            # buffer across rounds (cap bufs=4) and the PE stream never
            # stalls at a round boundary (keeps HAM at 8/8).
            cap = es.enter_context(tc.tile_pool(name="cap", bufs=4, space="PSUM"))

            rounds = [(jt, fb) for jt in range(2) for fb in range(S // FW)]

            # round 0: dense PV burst over the prologue's E tiles, with
            # round 1's first units interleaved so the exp stream keeps
            # running through the burst window
            chains0 = {}
            chains1 = {}
            for dd in range(2):
                chains0[dd] = cap.tile([65, FW], mybir.dt.float32, tag="cacc",
                                       name=f"cacc0_{dd}")
            for dd in range(2):
                chains1[dd] = cap.tile([65, FW], mybir.dt.float32, tag="cacc",
                                       name=f"cacc1_{dd}")
            u1 = 0
            for tt in range(NT):
                for dd in range(2):
                    nc.tensor.matmul(
                        chains0[dd][:], vp[tt][:, dd, :],
                        E0[tt][:, 512 * dd:512 * dd + FW],
                        start=(tt == 0), stop=(tt == NT - 1))
                if tt % 2 == 1 and u1 < NT // 2:
                    e = emit_st_exp(0, 1, u1)
                    for dd in range(2):
                        nc.tensor.matmul(
                            chains1[dd][:], vp[u1][:, dd, :],
                            e[:, 512 * dd:512 * dd + FW],
                            start=(u1 == 0), stop=False)
                    u1 += 1
            emit_recips(chains0, 0, 0, 0, pending)

            # round 1's tail plus rounds 2..7 as ONE flat unit stream,
            # software-pipelined with a one-unit skew: each unit emits
            # ST/exp for unit u, then the PVs for unit u-1. The PE then
            # always has a ready ST between a score matmul and the PV that
            # depends on its exp, removing the ~1us round-boundary stall
            # (which is also what trips the HAM clock throttle into its
            # sticky half-speed state).
            units = [(1, 0, 1, tt) for tt in range(u1, NT)]
            units += [(rnd, jt, fb, tt)
                      for rnd, (jt, fb) in enumerate(rounds[2:], start=2)
                      for tt in range(NT)]
            chains_by_rnd = {1: chains1}
            prevu = None

            def emit_pv(prnd, pjt, pfb, ptt, pe):
                for dd in range(2):
                    nc.tensor.matmul(
                        chains_by_rnd[prnd][dd][:],
                        vp[ptt][:, 2 * pjt + dd, :],
                        pe[:, 512 * dd:512 * dd + FW],
                        start=(ptt == 0), stop=(ptt == NT - 1))
                if ptt == NT - 1:
                    emit_recips(chains_by_rnd[prnd], pjt, FW * pfb, prnd,
                                pending, on_act=(prnd == len(rounds) - 1))

            for rnd, jt, fb, tt in units:
                if tt == 0:
                    chains_by_rnd[rnd] = {
                        dd: cap.tile([65, FW], mybir.dt.float32, tag="cacc",
                                     name=f"cacc{jt}_{fb}_{dd}")
                        for dd in range(2)}
                e = emit_st_exp(jt, fb, tt)
                if prevu is not None:
                    emit_pv(*prevu)
                if pending and tt in (5, 11):
                    emit_epilogue_tail(*pending.pop(0))
                prevu = (rnd, jt, fb, tt, e)
            emit_pv(*prevu)
            for args in pending:
                emit_epilogue_tail(*args)

    nc.compile()
    return nc


def _get_nc():
    global _cached
    if _cached is None:
        _cached = _build()
    return _cached


def _numpy_fallback(from_tensor, to_tensor, attention_mask, Wq, bq, Wk, bk, Wv, bv):
    b, f, _ = from_tensor.shape
    t = to_tensor.shape[1]
    h, d = NUM_HEADS, HEAD_DIM
    q = (from_tensor @ Wq + bq).reshape(b, f, h, d).transpose(0, 2, 1, 3)
    k = (to_tensor @ Wk + bk).reshape(b, t, h, d).transpose(0, 2, 1, 3)
    v = (to_tensor @ Wv + bv).reshape(b, t, h, d).transpose(0, 2, 1, 3)
    scores = np.einsum("bhfd,bhtd->bhft", q, k) * (1.0 / np.sqrt(float(d)))
    adder = (1.0 - attention_mask[:, None].astype(np.float32)) * -10000.0
    scores = scores + adder
    scores = scores - scores.max(axis=-1, keepdims=True)
    e = np.exp(scores)
    probs = e / e.sum(axis=-1, keepdims=True)
    ctx = np.einsum("bhft,bhtd->bhfd", probs, v)
    return ctx.transpose(0, 2, 1, 3).reshape(b, f, h * d).astype(np.float32)


def _make_in_maps(from_tensor, to_tensor, Wq, bq, Wk, bk, Wv, bv):
    import ml_dtypes
    bf16 = ml_dtypes.bfloat16
    fromT = [np.ascontiguousarray(from_tensor[b].T).astype(bf16) for b in range(B)]
    toT = [np.ascontiguousarray(to_tensor[b].T).astype(bf16) for b in range(B)]
    in_maps = []
    for core in range(8):
        b, g = divmod(core, G)
        j0 = JW * g
        in_maps.append({
            "fromT": fromT[b],
            "toT": toT[b],
            "wq": np.ascontiguousarray(Wq[:, j0:j0 + JW]).astype(bf16),
            "wk": np.ascontiguousarray(Wk[:, j0:j0 + JW]).astype(bf16),
            "wv": np.ascontiguousarray(Wv[:, j0:j0 + JW]).astype(bf16),
            "bq": np.ascontiguousarray(bq[j0:j0 + JW].reshape(JW, 1)),
            "bk": np.ascontiguousarray(bk[j0:j0 + JW].reshape(JW, 1)),
            "bv": np.ascontiguousarray(bv[j0:j0 + JW].reshape(1, JW)),
        })
    return in_maps


def profile_exec_time(inputs):
    """Rerun on HW with NTFF tracing; returns whole-NEFF exec time in ns."""
    from concourse import bass_utils
    nc = _get_nc()
    in_maps = _make_in_maps(
        np.asarray(inputs["from_tensor"], dtype=np.float32),
        np.asarray(inputs["to_tensor"], dtype=np.float32),
        np.asarray(inputs["Wq"], dtype=np.float32),
        np.asarray(inputs["bq"], dtype=np.float32),
        np.asarray(inputs["Wk"], dtype=np.float32),
        np.asarray(inputs["bk"], dtype=np.float32),
        np.asarray(inputs["Wv"], dtype=np.float32),
        np.asarray(inputs["bv"], dtype=np.float32))
    res = bass_utils.run_bass_kernel_spmd(nc, in_maps, core_ids=list(range(8)),
                                          trace=True)
    profile_exec_time.last_results = res
    return res.exec_time_ns


def kernel(**inputs) -> np.ndarray:
    from_tensor = np.asarray(inputs["from_tensor"], dtype=np.float32)
    to_tensor = np.asarray(inputs["to_tensor"], dtype=np.float32)
    attention_mask = np.asarray(inputs["attention_mask"])
    Wq = np.asarray(inputs["Wq"], dtype=np.float32)
    bq = np.asarray(inputs["bq"], dtype=np.float32)
    Wk = np.asarray(inputs["Wk"], dtype=np.float32)
    bk = np.asarray(inputs["bk"], dtype=np.float32)
    Wv = np.asarray(inputs["Wv"], dtype=np.float32)
    bv = np.asarray(inputs["bv"], dtype=np.float32)

    if not np.all(attention_mask == 1):
        # General-mask path (not exercised by the spec'd all-ones fill):
        # plain numpy reference math.
        return _numpy_fallback(from_tensor, to_tensor, attention_mask,
                               Wq, bq, Wk, bk, Wv, bv)

    from concourse import bass_utils

    nc = _get_nc()

    in_maps = _make_in_maps(from_tensor, to_tensor, Wq, bq, Wk, bk, Wv, bv)
    res = bass_utils.run_bass_kernel_spmd(nc, in_maps, core_ids=list(range(8)))
    kernel.last_results = res

    output = np.empty((B, S, HID), dtype=np.float32)
    for core in range(8):
        b, g = divmod(core, G)
        j0 = JW * g
        output[b, :, j0:j0 + JW] = res.results[core]["out"].T
    return output


if __name__ == "__main__":
    rng = np.random.default_rng(0)
    ins = {
        "from_tensor": rng.standard_normal((B, S, HID), dtype=np.float32),
        "to_tensor": rng.standard_normal((B, S, HID), dtype=np.float32),
        "attention_mask": np.ones((B, S, S), dtype=np.int32),
        "Wq": rng.standard_normal((HID, HID), dtype=np.float32) * 0.02,
        "bq": rng.standard_normal((HID,), dtype=np.float32) * 0.01,
        "Wk": rng.standard_normal((HID, HID), dtype=np.float32) * 0.02,
        "bk": rng.standard_normal((HID,), dtype=np.float32) * 0.01,
        "Wv": rng.standard_normal((HID, HID), dtype=np.float32) * 0.02,
        "bv": rng.standard_normal((HID,), dtype=np.float32) * 0.01,
    }
    got = kernel(**ins)
    want = _numpy_fallback(**ins)
    err = np.abs(got - want).max() / np.abs(want).max()
    print("self-test rel err:", err)


# revision 42
# speedup vs baseline: 1.0101x; 1.0101x over previous
"""Trainium2 Bass kernel for nn_AttentionLayer (B=2, S=2048, HID=1024, 16 heads x 64).

Sharding: 8 cores = 2 batches x 4 head-groups (4 heads each). Each core computes
its batch's attention for its 4 heads and writes a disjoint [256, 2048] slice of
the output (transposed). No collectives.

Device math (per core, all layouts feature-major to match TensorE):
  Q.T = Wq_g.T @ X_f.T + bq_g      [256 j, 2048 f]   (f32r matmuls)
  K.T = Wk_g.T @ X_t.T + bk_g      [256 j, 2048 t]
  V'  = [X_t.T.T @ Wv_g + 1*bv_g | ones]  [2048 t, 4, 65] (bf16)
  S.T = K.T_h.T @ Q.T_h            [t, f] per head    (row-packed pairs, K=64)
  E   = exp(S.T / 8)               (bf16, ScalarE)
  ctx'.T = V'_h.T @ E              [65, f]; row 64 = softmax denominator
  out = ctx'.T[0:64] * (1/denom broadcast)            (already includes bv)

Phase B is a software pipeline over units (tt, dd): ST (2 matmuls) -> exp
(1 ScalarE activation [128,1024]) -> PV (2 accumulating matmuls), with PSUM
split as 2 double-buffered ST slots (2 banks each) + 4 chain accumulators
(1 bank each). This keeps the PE free of >3.4us idle windows (HAM stays at
8/8) and keeps the ScalarE exp stream (the throughput floor) saturated.
"""
import numpy as np

B, S, HID = 2, 2048, 1024
NUM_HEADS, HEAD_DIM = 16, 64
G = 4                 # head-groups (cores per batch)
HPC = 4               # heads per core
JW = HPC * HEAD_DIM   # 256 W columns per core
NC_CHUNKS = HID // 128  # 8 contraction chunks
NT = S // 128         # 16 t tiles
NFB = 2               # f blocks of 1024
SCALE = 1.0 / np.sqrt(float(HEAD_DIM))

_cached = None


def _build():
    import contextlib
    import concourse.bass as bass
    import concourse.tile as tile
    from concourse import bacc, mybir

    F32R = mybir.dt.float32r
    F32 = mybir.dt.float32
    BF16 = mybir.dt.bfloat16
    Act = mybir.ActivationFunctionType

    nc = bacc.Bacc("TRN2", target_bir_lowering=False, debug=False, num_devices=8)

    fromT = nc.dram_tensor("fromT", (HID, S), BF16, kind="ExternalInput").ap()
    toT = nc.dram_tensor("toT", (HID, S), BF16, kind="ExternalInput").ap()
    wq = nc.dram_tensor("wq", (HID, JW), BF16, kind="ExternalInput").ap()
    wk = nc.dram_tensor("wk", (HID, JW), BF16, kind="ExternalInput").ap()
    wv = nc.dram_tensor("wv", (HID, JW), BF16, kind="ExternalInput").ap()
    bq = nc.dram_tensor("bq", (JW, 1), F32, kind="ExternalInput").ap()
    bk = nc.dram_tensor("bk", (JW, 1), F32, kind="ExternalInput").ap()
    bv = nc.dram_tensor("bv", (1, JW), F32R, kind="ExternalInput").ap()
    out = nc.dram_tensor("out", (JW, S), F32, kind="ExternalOutput").ap()

    with tile.TileContext(nc) as tc:
        with contextlib.ExitStack() as es:
            persist = es.enter_context(tc.tile_pool(name="persist", bufs=1))

            # --- constants / biases
            bq_sb = []
            bk_sb = []
            for jt in range(2):
                tq = persist.tile([128, 1], F32, tag=f"bq{jt}")
                nc.sync.dma_start(tq[:], bq[128 * jt:128 * jt + 128, 0:1])
                bq_sb.append(tq)
                tk = persist.tile([128, 1], F32, tag=f"bk{jt}")
                nc.sync.dma_start(tk[:], bk[128 * jt:128 * jt + 128, 0:1])
                bk_sb.append(tk)
            bv_row = persist.tile([1, JW], F32R, tag="bvrow")
            nc.sync.dma_start(bv_row[:], bv[0:1, :])
            ones_f = persist.tile([1, 128], F32, tag="onesf")
            nc.vector.memset(ones_f[:], 1.0)
            ones_r = persist.tile([1, 128], F32R, tag="onesr")
            nc.vector.tensor_copy(ones_r[:], ones_f[:])
            ones_bf = persist.tile([1, 64], BF16, tag="onesbf")
            nc.vector.memset(ones_bf[:], 1.0)

            # --- persistent projection outputs
            qt = [persist.tile([128, S], F32R, tag=f"qt{jt}", name=f"qt{jt}") for jt in range(2)]
            kt = [persist.tile([128, S], F32R, tag=f"kt{jt}", name=f"kt{jt}") for jt in range(2)]
            vp = [persist.tile([128, HPC, 65], BF16, tag=f"vp{tt}", name=f"vp{tt}") for tt in range(NT)]

            # Phase-B SBUF/PSUM pools are opened BEFORE the phase-A PSUM
            # pools (pool release is LIFO): stp(4 banks) coexists with
            # psA(2)+psAv(2) during phase A, then cap(4) opens once the
            # phase-A pools close. This lets round 0's ST+exp stream start
            # inside phase A, right after the K projections.
            epool = es.enter_context(tc.tile_pool(name="epool", bufs=18))
            spool = es.enter_context(tc.tile_pool(name="spool", bufs=4))
            stp = es.enter_context(tc.tile_pool(name="stp", bufs=2, space="PSUM"))

            FW = 512  # f-block width per round

            def emit_st_exp(jt, fb, tt):
                fo = FW * fb
                st = stp.tile([128, 1024], mybir.dt.float32,
                              tag="st", name=f"st{jt}_{fb}_{tt}")
                for dd in range(2):
                    nc.tensor.matmul(
                        st[:, 512 * dd:512 * dd + FW],
                        kt[jt][64 * dd:64 * dd + 64, 128 * tt:128 * tt + 128],
                        qt[jt][64 * dd:64 * dd + 64, fo:fo + FW],
                        start=True, stop=True,
                        tile_position=(64 * dd, 0))
                e = epool.tile([128, 1024], BF16, tag="E",
                               name=f"e{jt}_{fb}_{tt}")
                nc.scalar.activation(e[:], st[:], Act.Exp, bias=0.0, scale=SCALE)
                return e

            def emit_epilogue_tail(cacc, rcp, k_local, fo, rnd):
                # Broadcast 1/denom over 64 partitions via a tiny PE
                # matmul into a rotating ST slot, then scale + store.
                bt = stp.tile([128, 1024], mybir.dt.float32,
                              tag="st", name=f"bt{rnd}_{k_local}_{fo}")
                nc.tensor.matmul(bt[0:64, 0:FW], ones_bf[0:1, :], rcp[:],
                                 start=True, stop=True)
                bcs = spool.tile([64, FW], F32, tag="bcs")
                nc.vector.tensor_copy(bcs[:], bt[0:64, 0:FW])
                so = spool.tile([64, FW], F32, tag="so")
                nc.vector.tensor_mul(so[:], cacc[0:64, :], bcs[:])
                nc.sync.dma_start(
                    out[64 * k_local:64 * k_local + 64, fo:fo + FW],
                    so[:])

            def emit_recips(chains, jt, fo, rnd, pending, on_act=False):
                for dd in range(2):
                    cacc = chains[dd]
                    rcp = spool.tile([1, FW], BF16, tag="rcp")
                    if on_act:
                        # drain-tail path: 1/d = exp(-ln d) on the (idle)
                        # ScalarE -- ~2x faster than the serial DVE recips
                        lnt = spool.tile([1, FW], F32, tag="lnt")
                        nc.scalar.activation(lnt[:], cacc[64:65, :], Act.Ln,
                                             bias=0.0, scale=1.0)
                        nc.scalar.activation(rcp[:], lnt[:], Act.Exp,
                                             bias=0.0, scale=-1.0)
                    else:
                        with nc.allow_low_precision(reason="softmax recip; tol 2e-2"):
                            nc.vector.reciprocal(rcp[:], cacc[64:65, :])
                    pending.append((cacc, rcp, 2 * jt + dd, fo, rnd))

            pending = []

            # --- phase A: projections (+ round-0 ST/exp prologue)
            E0 = []
            with contextlib.ExitStack() as esA:
                pA = esA.enter_context(tc.tile_pool(name="phaseA", bufs=1))
                psA = esA.enter_context(tc.tile_pool(name="psA", bufs=2, space="PSUM"))
                psAv = esA.enter_context(tc.tile_pool(name="psAv", bufs=2, space="PSUM"))
                wq_sb = pA.tile([128, NC_CHUNKS, JW], BF16, tag="wq")
                nc.sync.dma_start(wq_sb[:], wq.rearrange("(c p) j -> p c j", p=128))
                wk_sb = pA.tile([128, NC_CHUNKS, JW], BF16, tag="wk")
                nc.gpsimd.dma_start(wk_sb[:], wk.rearrange("(c p) j -> p c j", p=128))
                wv_sb = pA.tile([128, NC_CHUNKS, JW], BF16, tag="wv")
                nc.scalar.dma_start(wv_sb[:], wv.rearrange("(c p) j -> p c j", p=128))

                # stream fromT/toT as full-row [128, S] bf16 chunk tiles
                # (4KB/partition, fully contiguous DRAM reads), DMAs spread
                # across the three DMA-capable engine queues.
                dma_engines = [nc.sync, nc.gpsimd, nc.scalar]

                def stream(src, nm):
                    tiles = []
                    for c in range(NC_CHUNKS):
                        t = pA.tile([128, S], BF16, tag="xc",
                                    name=f"x{nm}{c}", bufs=16)
                        dma_engines[c % 3].dma_start(
                            t[:], src[128 * c:128 * c + 128, :])
                        tiles.append(t)
                    return tiles

                fx = stream(fromT, "f")
                tx = stream(toT, "t")

                for jt in range(2):
                    for fc in range(4):
                        acc = psA.tile([128, 512], mybir.dt.float32, tag="big")
                        for c in range(NC_CHUNKS):
                            nc.tensor.matmul(
                                acc[:],
                                wq_sb[:, c, 128 * jt:128 * jt + 128],
                                fx[c][:, 512 * fc:512 * fc + 512],
                                start=(c == 0), stop=(c == NC_CHUNKS - 1))
                        nc.vector.tensor_scalar_add(
                            qt[jt][:, 512 * fc:512 * fc + 512], acc[:],
                            bq_sb[jt][:])
                # K projections, with the round-0 ST/exp prologue interleaved
                # per jt=0 column block: the exp stream (the throughput
                # floor) starts as soon as the first K chain lands instead of
                # after all eight.
                for jt in range(2):
                    for fc in range(4):
                        acc = psA.tile([128, 512], mybir.dt.float32, tag="big")
                        for c in range(NC_CHUNKS):
                            nc.tensor.matmul(
                                acc[:],
                                wk_sb[:, c, 128 * jt:128 * jt + 128],
                                tx[c][:, 512 * fc:512 * fc + 512],
                                start=(c == 0), stop=(c == NC_CHUNKS - 1))
                        nc.vector.tensor_scalar_add(
                            kt[jt][:, 512 * fc:512 * fc + 512], acc[:],
                            bk_sb[jt][:])
                        if jt == 0:
                            for tt in range(4 * fc, 4 * fc + 4):
                                E0.append(emit_st_exp(0, 0, tt))

                for tt in range(NT):
                    accv = psAv.tile([128, 512], mybir.dt.float32, tag="sm")
                    nc.tensor.matmul(accv[:, 0:JW], ones_r[0:1, :], bv_row[0:1, :],
                                     start=True, stop=False)
                    for c in range(NC_CHUNKS):
                        nc.tensor.matmul(
                            accv[:, 0:JW],
                            tx[c][:, 128 * tt:128 * tt + 128],
                            wv_sb[:, c, :],
                            start=False, stop=(c == NC_CHUNKS - 1))
                    nc.vector.memset(vp[tt][:, :, 64], 1.0)
                    nc.vector.tensor_copy(
                        vp[tt][:, :, 0:64],
                        accv[:, 0:JW].rearrange("p (k e) -> p k e", k=HPC))

            # --- phase B: attention, software-pipelined per tt unit.
            # Rounds are (jt, fb) with 512-wide f-blocks so each round only
            # holds 2 chain accumulators (1 PSUM bank each) -> chains double-
            # buffer across rounds (cap bufs=4) and the PE stream never
            # stalls at a round boundary (keeps HAM at 8/8).
            cap = es.enter_context(tc.tile_pool(name="cap", bufs=4, space="PSUM"))

            rounds = [(jt, fb) for jt in range(2) for fb in range(S // FW)]

            # round 0: dense PV burst over the prologue's E tiles, with
            # round 1's first units interleaved so the exp stream keeps
            # running through the burst window
            chains0 = {}
            chains1 = {}
            for dd in range(2):
                chains0[dd] = cap.tile([65, FW], mybir.dt.float32, tag="cacc",
                                       name=f"cacc0_{dd}")
            for dd in range(2):
                chains1[dd] = cap.tile([65, FW], mybir.dt.float32, tag="cacc",
                                       name=f"cacc1_{dd}")
            u1 = 0
            for tt in range(NT):
                for dd in range(2):
                    nc.tensor.matmul(
                        chains0[dd][:], vp[tt][:, dd, :],
                        E0[tt][:, 512 * dd:512 * dd + FW],
                        start=(tt == 0), stop=(tt == NT - 1))
                if tt % 2 == 1 and u1 < NT // 2:
                    e = emit_st_exp(0, 1, u1)
                    for dd in range(2):
                        nc.tensor.matmul(
                            chains1[dd][:], vp[u1][:, dd, :],
                            e[:, 512 * dd:512 * dd + FW],
                            start=(u1 == 0), stop=False)
                    u1 += 1
            emit_recips(chains0, 0, 0, 0, pending)

            # round 1's tail plus rounds 2..7 as ONE flat unit stream,
            # software-pipelined with a one-unit skew: each unit emits
            # ST/exp for unit u, then the PVs for unit u-1. The PE then
            # always has a ready ST between a score matmul and the PV that
            # depends on its exp, removing the ~1us round-boundary stall
            # (which is also what trips the HAM clock throttle into its
            # sticky half-speed state).
            units = [(1, 0, 1, tt) for tt in range(u1, NT)]
            units += [(rnd, jt, fb, tt)
                      for rnd, (jt, fb) in enumerate(rounds[2:], start=2)
                      for tt in range(NT)]
            chains_by_rnd = {1: chains1}
            prevu = None

            def emit_pv(prnd, pjt, pfb, ptt, pe):
                for dd in range(2):
                    nc.tensor.matmul(
                        chains_by_rnd[prnd][dd][:],
                        vp[ptt][:, 2 * pjt + dd, :],
                        pe[:, 512 * dd:512 * dd + FW],
                        start=(ptt == 0), stop=(ptt == NT - 1))
                if ptt == NT - 1:
                    emit_recips(chains_by_rnd[prnd], pjt, FW * pfb, prnd,
                                pending, on_act=(prnd == len(rounds) - 1))

            for rnd, jt, fb, tt in units:
                if tt == 0:
                    chains_by_rnd[rnd] = {
                        dd: cap.tile([65, FW], mybir.dt.float32, tag="cacc",
                                     name=f"cacc{jt}_{fb}_{dd}")
                        for dd in range(2)}
                e = emit_st_exp(jt, fb, tt)
                if prevu is not None:
                    emit_pv(*prevu)
                if pending and tt in (5, 11):
                    emit_epilogue_tail(*pending.pop(0))
                prevu = (rnd, jt, fb, tt, e)
            emit_pv(*prevu)
            for args in pending:
                emit_epilogue_tail(*args)

    nc.compile()
    return nc


def _get_nc():
    global _cached
    if _cached is None:
        _cached = _build()
    return _cached


def _numpy_fallback(from_tensor, to_tensor, attention_mask, Wq, bq, Wk, bk, Wv, bv):
    b, f, _ = from_tensor.shape
    t = to_tensor.shape[1]
    h, d = NUM_HEADS, HEAD_DIM
    q = (from_tensor @ Wq + bq).reshape(b, f, h, d).transpose(0, 2, 1, 3)
    k = (to_tensor @ Wk + bk).reshape(b, t, h, d).transpose(0, 2, 1, 3)
    v = (to_tensor @ Wv + bv).reshape(b, t, h, d).transpose(0, 2, 1, 3)
    scores = np.einsum("bhfd,bhtd->bhft", q, k) * (1.0 / np.sqrt(float(d)))
    adder = (1.0 - attention_mask[:, None].astype(np.float32)) * -10000.0
    scores = scores + adder
    scores = scores - scores.max(axis=-1, keepdims=True)
    e = np.exp(scores)
    probs = e / e.sum(axis=-1, keepdims=True)
    ctx = np.einsum("bhft,bhtd->bhfd", probs, v)
    return ctx.transpose(0, 2, 1, 3).reshape(b, f, h * d).astype(np.float32)


def _make_in_maps(from_tensor, to_tensor, Wq, bq, Wk, bk, Wv, bv):
    import ml_dtypes
    bf16 = ml_dtypes.bfloat16
    fromT = [np.ascontiguousarray(from_tensor[b].T).astype(bf16) for b in range(B)]
    toT = [np.ascontiguousarray(to_tensor[b].T).astype(bf16) for b in range(B)]
    in_maps = []
    for core in range(8):
        b, g = divmod(core, G)
        j0 = JW * g
        in_maps.append({
            "fromT": fromT[b],
            "toT": toT[b],
            "wq": np.ascontiguousarray(Wq[:, j0:j0 + JW]).astype(bf16),
            "wk": np.ascontiguousarray(Wk[:, j0:j0 + JW]).astype(bf16),
            "wv": np.ascontiguousarray(Wv[:, j0:j0 + JW]).astype(bf16),
            "bq": np.ascontiguousarray(bq[j0:j0 + JW].reshape(JW, 1)),
            "bk": np.ascontiguousarray(bk[j0:j0 + JW].reshape(JW, 1)),
            "bv": np.ascontiguousarray(bv[j0:j0 + JW].reshape(1, JW)),
        })
    return in_maps


def profile_exec_time(inputs):
    """Rerun on HW with NTFF tracing; returns whole-NEFF exec time in ns."""
    from concourse import bass_utils
    nc = _get_nc()
    in_maps = _make_in_maps(
        np.asarray(inputs["from_tensor"], dtype=np.float32),
        np.asarray(inputs["to_tensor"], dtype=np.float32),
        np.asarray(inputs["Wq"], dtype=np.float32),
        np.asarray(inputs["bq"], dtype=np.float32),
        np.asarray(inputs["Wk"], dtype=np.float32),
        np.asarray(inputs["bk"], dtype=np.float32),
        np.asarray(inputs["Wv"], dtype=np.float32),
        np.asarray(inputs["bv"], dtype=np.float32))
    res = bass_utils.run_bass_kernel_spmd(nc, in_maps, core_ids=list(range(8)),
                                          trace=True)
    profile_exec_time.last_results = res
    return res.exec_time_ns


def kernel(**inputs) -> np.ndarray:
    from_tensor = np.asarray(inputs["from_tensor"], dtype=np.float32)
    to_tensor = np.asarray(inputs["to_tensor"], dtype=np.float32)
    attention_mask = np.asarray(inputs["attention_mask"])
    Wq = np.asarray(inputs["Wq"], dtype=np.float32)
    bq = np.asarray(inputs["bq"], dtype=np.float32)
    Wk = np.asarray(inputs["Wk"], dtype=np.float32)
    bk = np.asarray(inputs["bk"], dtype=np.float32)
    Wv = np.asarray(inputs["Wv"], dtype=np.float32)
    bv = np.asarray(inputs["bv"], dtype=np.float32)

    if not np.all(attention_mask == 1):
        # General-mask path (not exercised by the spec'd all-ones fill):
        # plain numpy reference math.
        return _numpy_fallback(from_tensor, to_tensor, attention_mask,
                               Wq, bq, Wk, bk, Wv, bv)

    from concourse import bass_utils

    nc = _get_nc()

    in_maps = _make_in_maps(from_tensor, to_tensor, Wq, bq, Wk, bk, Wv, bv)
    res = bass_utils.run_bass_kernel_spmd(nc, in_maps, core_ids=list(range(8)))
    kernel.last_results = res

    output = np.empty((B, S, HID), dtype=np.float32)
    for core in range(8):
        b, g = divmod(core, G)
        j0 = JW * g
        output[b, :, j0:j0 + JW] = res.results[core]["out"].T
    return output


if __name__ == "__main__":
    rng = np.random.default_rng(0)
    ins = {
        "from_tensor": rng.standard_normal((B, S, HID), dtype=np.float32),
        "to_tensor": rng.standard_normal((B, S, HID), dtype=np.float32),
        "attention_mask": np.ones((B, S, S), dtype=np.int32),
        "Wq": rng.standard_normal((HID, HID), dtype=np.float32) * 0.02,
        "bq": rng.standard_normal((HID,), dtype=np.float32) * 0.01,
        "Wk": rng.standard_normal((HID, HID), dtype=np.float32) * 0.02,
        "bk": rng.standard_normal((HID,), dtype=np.float32) * 0.01,
        "Wv": rng.standard_normal((HID, HID), dtype=np.float32) * 0.02,
        "bv": rng.standard_normal((HID,), dtype=np.float32) * 0.01,
    }
    got = kernel(**ins)
    want = _numpy_fallback(**ins)
    err = np.abs(got - want).max() / np.abs(want).max()
    print("self-test rel err:", err)


# revision 43
# speedup vs baseline: 1.0333x; 1.0230x over previous
"""Trainium2 Bass kernel for nn_AttentionLayer (B=2, S=2048, HID=1024, 16 heads x 64).

Sharding: 8 cores = 2 batches x 4 head-groups (4 heads each). Each core computes
its batch's attention for its 4 heads and writes a disjoint [256, 2048] slice of
the output (transposed). No collectives.

Device math (per core, all layouts feature-major to match TensorE):
  Q.T = Wq_g.T @ X_f.T + bq_g      [256 j, 2048 f]   (f32r matmuls)
  K.T = Wk_g.T @ X_t.T + bk_g      [256 j, 2048 t]
  V'  = [X_t.T.T @ Wv_g + 1*bv_g | ones]  [2048 t, 4, 65] (bf16)
  S.T = K.T_h.T @ Q.T_h            [t, f] per head    (row-packed pairs, K=64)
  E   = exp(S.T / 8)               (bf16, ScalarE)
  ctx'.T = V'_h.T @ E              [65, f]; row 64 = softmax denominator
  out = ctx'.T[0:64] * (1/denom broadcast)            (already includes bv)

Phase B is a software pipeline over units (tt, dd): ST (2 matmuls) -> exp
(1 ScalarE activation [128,1024]) -> PV (2 accumulating matmuls), with PSUM
split as 2 double-buffered ST slots (2 banks each) + 4 chain accumulators
(1 bank each). This keeps the PE free of >3.4us idle windows (HAM stays at
8/8) and keeps the ScalarE exp stream (the throughput floor) saturated.
"""
import numpy as np

B, S, HID = 2, 2048, 1024
NUM_HEADS, HEAD_DIM = 16, 64
G = 4                 # head-groups (cores per batch)
HPC = 4               # heads per core
JW = HPC * HEAD_DIM   # 256 W columns per core
NC_CHUNKS = HID // 128  # 8 contraction chunks
NT = S // 128         # 16 t tiles
NFB = 2               # f blocks of 1024
SCALE = 1.0 / np.sqrt(float(HEAD_DIM))

_cached = None


def _build():
    import contextlib
    import concourse.bass as bass
    import concourse.tile as tile
    from concourse import bacc, mybir

    F32R = mybir.dt.float32r
    F32 = mybir.dt.float32
    BF16 = mybir.dt.bfloat16
    Act = mybir.ActivationFunctionType

    nc = bacc.Bacc("TRN2", target_bir_lowering=False, debug=False, num_devices=8)

    fromT = nc.dram_tensor("fromT", (HID, S), BF16, kind="ExternalInput").ap()
    toT = nc.dram_tensor("toT", (HID, S), BF16, kind="ExternalInput").ap()
    wq = nc.dram_tensor("wq", (HID, JW), BF16, kind="ExternalInput").ap()
    wk = nc.dram_tensor("wk", (HID, JW), BF16, kind="ExternalInput").ap()
    wv = nc.dram_tensor("wv", (HID, JW), BF16, kind="ExternalInput").ap()
    bq = nc.dram_tensor("bq", (JW, 1), F32, kind="ExternalInput").ap()
    bk = nc.dram_tensor("bk", (JW, 1), F32, kind="ExternalInput").ap()
    bv = nc.dram_tensor("bv", (1, JW), F32R, kind="ExternalInput").ap()
    out = nc.dram_tensor("out", (JW, S), F32, kind="ExternalOutput").ap()

    with tile.TileContext(nc) as tc:
        with contextlib.ExitStack() as es:
            persist = es.enter_context(tc.tile_pool(name="persist", bufs=1))

            # --- constants / biases
            bq_sb = []
            bk_sb = []
            for jt in range(2):
                tq = persist.tile([128, 1], F32, tag=f"bq{jt}")
                nc.sync.dma_start(tq[:], bq[128 * jt:128 * jt + 128, 0:1])
                bq_sb.append(tq)
                tk = persist.tile([128, 1], F32, tag=f"bk{jt}")
                nc.sync.dma_start(tk[:], bk[128 * jt:128 * jt + 128, 0:1])
                bk_sb.append(tk)
            bv_row = persist.tile([1, JW], F32R, tag="bvrow")
            nc.sync.dma_start(bv_row[:], bv[0:1, :])
            ones_f = persist.tile([1, 128], F32, tag="onesf")
            nc.vector.memset(ones_f[:], 1.0)
            ones_r = persist.tile([1, 128], F32R, tag="onesr")
            nc.vector.tensor_copy(ones_r[:], ones_f[:])
            ones_bf = persist.tile([1, 64], BF16, tag="onesbf")
            nc.vector.memset(ones_bf[:], 1.0)

            # --- persistent projection outputs
            qt = [persist.tile([128, S], F32R, tag=f"qt{jt}", name=f"qt{jt}") for jt in range(2)]
            kt = [persist.tile([128, S], F32R, tag=f"kt{jt}", name=f"kt{jt}") for jt in range(2)]
            vp = [persist.tile([128, HPC, 65], BF16, tag=f"vp{tt}", name=f"vp{tt}") for tt in range(NT)]

            # Phase-B SBUF/PSUM pools are opened BEFORE the phase-A PSUM
            # pools (pool release is LIFO): stp(4 banks) coexists with
            # psA(2)+psAv(2) during phase A, then cap(4) opens once the
            # phase-A pools close. This lets round 0's ST+exp stream start
            # inside phase A, right after the K projections.
            epool = es.enter_context(tc.tile_pool(name="epool", bufs=18))
            spool = es.enter_context(tc.tile_pool(name="spool", bufs=4))
            stp = es.enter_context(tc.tile_pool(name="stp", bufs=2, space="PSUM"))

            FW = 512  # f-block width per round

            def emit_st_exp(jt, fb, tt):
                fo = FW * fb
                st = stp.tile([128, 1024], mybir.dt.float32,
                              tag="st", name=f"st{jt}_{fb}_{tt}")
                for dd in range(2):
                    nc.tensor.matmul(
                        st[:, 512 * dd:512 * dd + FW],
                        kt[jt][64 * dd:64 * dd + 64, 128 * tt:128 * tt + 128],
                        qt[jt][64 * dd:64 * dd + 64, fo:fo + FW],
                        start=True, stop=True,
                        tile_position=(64 * dd, 0))
                e = epool.tile([128, 1024], BF16, tag="E",
                               name=f"e{jt}_{fb}_{tt}")
                nc.scalar.activation(e[:], st[:], Act.Exp, bias=0.0, scale=SCALE)
                return e

            def emit_epilogue_tail(cacc, rcp, k_local, fo, rnd):
                # Broadcast 1/denom over 64 partitions via a tiny PE
                # matmul into a rotating ST slot, then scale + store.
                bt = stp.tile([128, 1024], mybir.dt.float32,
                              tag="st", name=f"bt{rnd}_{k_local}_{fo}")
                nc.tensor.matmul(bt[0:64, 0:FW], ones_bf[0:1, :], rcp[:],
                                 start=True, stop=True)
                bcs = spool.tile([64, FW], F32, tag="bcs")
                nc.vector.tensor_copy(bcs[:], bt[0:64, 0:FW])
                so = spool.tile([64, FW], F32, tag="so")
                nc.vector.tensor_mul(so[:], cacc[0:64, :], bcs[:])
                nc.sync.dma_start(
                    out[64 * k_local:64 * k_local + 64, fo:fo + FW],
                    so[:])

            def emit_recips(chains, jt, fo, rnd, pending, on_act=False):
                for dd in range(2):
                    cacc = chains[dd]
                    rcp = spool.tile([1, FW], BF16, tag="rcp")
                    if on_act:
                        # drain-tail path: 1/d = exp(-ln d) on the (idle)
                        # ScalarE -- ~2x faster than the serial DVE recips
                        lnt = spool.tile([1, FW], F32, tag="lnt")
                        nc.scalar.activation(lnt[:], cacc[64:65, :], Act.Ln,
                                             bias=0.0, scale=1.0)
                        nc.scalar.activation(rcp[:], lnt[:], Act.Exp,
                                             bias=0.0, scale=-1.0)
                    else:
                        with nc.allow_low_precision(reason="softmax recip; tol 2e-2"):
                            nc.vector.reciprocal(rcp[:], cacc[64:65, :])
                    pending.append((cacc, rcp, 2 * jt + dd, fo, rnd))

            pending = []

            # --- phase A: projections (+ round-0 ST/exp prologue)
            E0 = []
            with contextlib.ExitStack() as esA:
                pA = esA.enter_context(tc.tile_pool(name="phaseA", bufs=1))
                psA = esA.enter_context(tc.tile_pool(name="psA", bufs=2, space="PSUM"))
                psAv = esA.enter_context(tc.tile_pool(name="psAv", bufs=2, space="PSUM"))
                wq_sb = pA.tile([128, NC_CHUNKS, JW], BF16, tag="wq")
                nc.sync.dma_start(wq_sb[:], wq.rearrange("(c p) j -> p c j", p=128))
                wk_sb = pA.tile([128, NC_CHUNKS, JW], BF16, tag="wk")
                nc.gpsimd.dma_start(wk_sb[:], wk.rearrange("(c p) j -> p c j", p=128))
                wv_sb = pA.tile([128, NC_CHUNKS, JW], BF16, tag="wv")
                nc.scalar.dma_start(wv_sb[:], wv.rearrange("(c p) j -> p c j", p=128))

                # stream fromT/toT as full-row [128, S] bf16 chunk tiles
                # (4KB/partition, fully contiguous DRAM reads), DMAs spread
                # across the three DMA-capable engine queues.
                dma_engines = [nc.sync, nc.gpsimd, nc.scalar]

                def stream(src, nm):
                    tiles = []
                    for c in range(NC_CHUNKS):
                        t = pA.tile([128, S], BF16, tag="xc",
                                    name=f"x{nm}{c}", bufs=16)
                        dma_engines[c % 3].dma_start(
                            t[:], src[128 * c:128 * c + 128, :])
                        tiles.append(t)
                    return tiles

                fx = stream(fromT, "f")
                tx = stream(toT, "t")

                for jt in range(2):
                    for fc in range(4):
                        acc = psA.tile([128, 512], mybir.dt.float32, tag="big")
                        for c in range(NC_CHUNKS):
                            nc.tensor.matmul(
                                acc[:],
                                wq_sb[:, c, 128 * jt:128 * jt + 128],
                                fx[c][:, 512 * fc:512 * fc + 512],
                                start=(c == 0), stop=(c == NC_CHUNKS - 1))
                        nc.vector.tensor_scalar_add(
                            qt[jt][:, 512 * fc:512 * fc + 512], acc[:],
                            bq_sb[jt][:])
                # K projections, with the round-0 ST/exp prologue interleaved
                # per jt=0 column block: the exp stream (the throughput
                # floor) starts as soon as the first K chain lands instead of
                # after all eight.
                for jt in range(2):
                    for fc in range(4):
                        acc = psA.tile([128, 512], mybir.dt.float32, tag="big")
                        for c in range(NC_CHUNKS):
                            nc.tensor.matmul(
                                acc[:],
                                wk_sb[:, c, 128 * jt:128 * jt + 128],
                                tx[c][:, 512 * fc:512 * fc + 512],
                                start=(c == 0), stop=(c == NC_CHUNKS - 1))
                        nc.vector.tensor_scalar_add(
                            kt[jt][:, 512 * fc:512 * fc + 512], acc[:],
                            bk_sb[jt][:])
                        if jt == 0:
                            for tt in range(4 * fc, 4 * fc + 4):
                                E0.append(emit_st_exp(0, 0, tt))

                for tt in range(NT):
                    accv = psAv.tile([128, 512], mybir.dt.float32, tag="sm")
                    nc.tensor.matmul(accv[:, 0:JW], ones_r[0:1, :], bv_row[0:1, :],
                                     start=True, stop=False)
                    for c in range(NC_CHUNKS):
                        nc.tensor.matmul(
                            accv[:, 0:JW],
                            tx[c][:, 128 * tt:128 * tt + 128],
                            wv_sb[:, c, :],
                            start=False, stop=(c == NC_CHUNKS - 1))
                    nc.vector.memset(vp[tt][:, :, 64], 1.0)
                    nc.vector.tensor_copy(
                        vp[tt][:, :, 0:64],
                        accv[:, 0:JW].rearrange("p (k e) -> p k e", k=HPC))

            # --- phase B: attention, software-pipelined per tt unit.
            # Rounds are (jt, fb) with 512-wide f-blocks so each round only
            # holds 2 chain accumulators (1 PSUM bank each) -> chains double-
            # buffer across rounds (cap bufs=4) and the PE stream never
            # stalls at a round boundary (keeps HAM at 8/8).
            cap = es.enter_context(tc.tile_pool(name="cap", bufs=4, space="PSUM"))

            rounds = [(jt, fb) for jt in range(2) for fb in range(S // FW)]

            # round 0: dense PV burst over the prologue's E tiles, with
            # round 1's first units interleaved so the exp stream keeps
            # running through the burst window
            chains0 = {}
            chains1 = {}
            for dd in range(2):
                chains0[dd] = cap.tile([65, FW], mybir.dt.float32, tag="cacc",
                                       name=f"cacc0_{dd}")
            for dd in range(2):
                chains1[dd] = cap.tile([65, FW], mybir.dt.float32, tag="cacc",
                                       name=f"cacc1_{dd}")
            u1 = 0
            for tt in range(NT):
                for dd in range(2):
                    nc.tensor.matmul(
                        chains0[dd][:], vp[tt][:, dd, :],
                        E0[tt][:, 512 * dd:512 * dd + FW],
                        start=(tt == 0), stop=(tt == NT - 1))
                if tt % 2 == 1 and u1 < NT // 2:
                    e = emit_st_exp(0, 1, u1)
                    for dd in range(2):
                        nc.tensor.matmul(
                            chains1[dd][:], vp[u1][:, dd, :],
                            e[:, 512 * dd:512 * dd + FW],
                            start=(u1 == 0), stop=False)
                    u1 += 1
            emit_recips(chains0, 0, 0, 0, pending)
            for tt in range(u1, NT):
                e = emit_st_exp(0, 1, tt)
                for dd in range(2):
                    nc.tensor.matmul(
                        chains1[dd][:], vp[tt][:, dd, :],
                        e[:, 512 * dd:512 * dd + FW],
                        start=(tt == 0), stop=(tt == NT - 1))
                if pending and tt in (11, 14):
                    emit_epilogue_tail(*pending.pop(0))
            emit_recips(chains1, 0, FW, 1, pending)

            # rounds 2..7 as ONE flat unit stream, software-pipelined with a
            # one-unit skew: each unit emits ST/exp for unit u, then the PVs
            # for unit u-1. The PE then always has a ready ST between a
            # score matmul and the PV that depends on its exp, removing the
            # ~1us boundary stall (which is also what trips the HAM clock
            # throttle into its sticky half-speed state).
            units = [(rnd, jt, fb, tt)
                     for rnd, (jt, fb) in enumerate(rounds[2:], start=2)
                     for tt in range(NT)]
            chains_by_rnd = {}
            prevu = None

            def emit_pv(prnd, pjt, pfb, ptt, pe):
                for dd in range(2):
                    nc.tensor.matmul(
                        chains_by_rnd[prnd][dd][:],
                        vp[ptt][:, 2 * pjt + dd, :],
                        pe[:, 512 * dd:512 * dd + FW],
                        start=(ptt == 0), stop=(ptt == NT - 1))
                if ptt == NT - 1:
                    emit_recips(chains_by_rnd[prnd], pjt, FW * pfb, prnd,
                                pending, on_act=(prnd == len(rounds) - 1))

            for rnd, jt, fb, tt in units:
                if tt == 0:
                    chains_by_rnd[rnd] = {
                        dd: cap.tile([65, FW], mybir.dt.float32, tag="cacc",
                                     name=f"cacc{jt}_{fb}_{dd}")
                        for dd in range(2)}
                e = emit_st_exp(jt, fb, tt)
                if prevu is not None:
                    emit_pv(*prevu)
                if pending and tt in (5, 11):
                    emit_epilogue_tail(*pending.pop(0))
                prevu = (rnd, jt, fb, tt, e)
            emit_pv(*prevu)
            for args in pending:
                emit_epilogue_tail(*args)

    nc.compile()
    return nc


def _get_nc():
    global _cached
    if _cached is None:
        _cached = _build()
    return _cached


def _numpy_fallback(from_tensor, to_tensor, attention_mask, Wq, bq, Wk, bk, Wv, bv):
    b, f, _ = from_tensor.shape
    t = to_tensor.shape[1]
    h, d = NUM_HEADS, HEAD_DIM
    q = (from_tensor @ Wq + bq).reshape(b, f, h, d).transpose(0, 2, 1, 3)
    k = (to_tensor @ Wk + bk).reshape(b, t, h, d).transpose(0, 2, 1, 3)
    v = (to_tensor @ Wv + bv).reshape(b, t, h, d).transpose(0, 2, 1, 3)
    scores = np.einsum("bhfd,bhtd->bhft", q, k) * (1.0 / np.sqrt(float(d)))
    adder = (1.0 - attention_mask[:, None].astype(np.float32)) * -10000.0
    scores = scores + adder
    scores = scores - scores.max(axis=-1, keepdims=True)
    e = np.exp(scores)
    probs = e / e.sum(axis=-1, keepdims=True)
    ctx = np.einsum("bhft,bhtd->bhfd", probs, v)
    return ctx.transpose(0, 2, 1, 3).reshape(b, f, h * d).astype(np.float32)


def _make_in_maps(from_tensor, to_tensor, Wq, bq, Wk, bk, Wv, bv):
    import ml_dtypes
    bf16 = ml_dtypes.bfloat16
    fromT = [np.ascontiguousarray(from_tensor[b].T).astype(bf16) for b in range(B)]
    toT = [np.ascontiguousarray(to_tensor[b].T).astype(bf16) for b in range(B)]
    in_maps = []
    for core in range(8):
        b, g = divmod(core, G)
        j0 = JW * g
        in_maps.append({
            "fromT": fromT[b],
            "toT": toT[b],
            "wq": np.ascontiguousarray(Wq[:, j0:j0 + JW]).astype(bf16),
            "wk": np.ascontiguousarray(Wk[:, j0:j0 + JW]).astype(bf16),
            "wv": np.ascontiguousarray(Wv[:, j0:j0 + JW]).astype(bf16),
            "bq": np.ascontiguousarray(bq[j0:j0 + JW].reshape(JW, 1)),
            "bk": np.ascontiguousarray(bk[j0:j0 + JW].reshape(JW, 1)),
            "bv": np.ascontiguousarray(bv[j0:j0 + JW].reshape(1, JW)),
        })
    return in_maps


def profile_exec_time(inputs):
    """Rerun on HW with NTFF tracing; returns whole-NEFF exec time in ns."""
    from concourse import bass_utils
    nc = _get_nc()
    in_maps = _make_in_maps(
        np.asarray(inputs["from_tensor"], dtype=np.float32),
        np.asarray(inputs["to_tensor"], dtype=np.float32),
        np.asarray(inputs["Wq"], dtype=np.float32),
        np.asarray(inputs["bq"], dtype=np.float32),
        np.asarray(inputs["Wk"], dtype=np.float32),
        np.asarray(inputs["bk"], dtype=np.float32),
        np.asarray(inputs["Wv"], dtype=np.float32),
        np.asarray(inputs["bv"], dtype=np.float32))
    res = bass_utils.run_bass_kernel_spmd(nc, in_maps, core_ids=list(range(8)),
                                          trace=True)
    profile_exec_time.last_results = res
    return res.exec_time_ns


def kernel(**inputs) -> np.ndarray:
    from_tensor = np.asarray(inputs["from_tensor"], dtype=np.float32)
    to_tensor = np.asarray(inputs["to_tensor"], dtype=np.float32)
    attention_mask = np.asarray(inputs["attention_mask"])
    Wq = np.asarray(inputs["Wq"], dtype=np.float32)
    bq = np.asarray(inputs["bq"], dtype=np.float32)
    Wk = np.asarray(inputs["Wk"], dtype=np.float32)
    bk = np.asarray(inputs["bk"], dtype=np.float32)
    Wv = np.asarray(inputs["Wv"], dtype=np.float32)
    bv = np.asarray(inputs["bv"], dtype=np.float32)

    if not np.all(attention_mask == 1):
        # General-mask path (not exercised by the spec'd all-ones fill):
        # plain numpy reference math.
        return _numpy_fallback(from_tensor, to_tensor, attention_mask,
                               Wq, bq, Wk, bk, Wv, bv)

    from concourse import bass_utils

    nc = _get_nc()

    in_maps = _make_in_maps(from_tensor, to_tensor, Wq, bq, Wk, bk, Wv, bv)
    res = bass_utils.run_bass_kernel_spmd(nc, in_maps, core_ids=list(range(8)))
    kernel.last_results = res

    output = np.empty((B, S, HID), dtype=np.float32)
    for core in range(8):
        b, g = divmod(core, G)
        j0 = JW * g
        output[b, :, j0:j0 + JW] = res.results[core]["out"].T
    return output


if __name__ == "__main__":
    rng = np.random.default_rng(0)
    ins = {
        "from_tensor": rng.standard_normal((B, S, HID), dtype=np.float32),
        "to_tensor": rng.standard_normal((B, S, HID), dtype=np.float32),
        "attention_mask": np.ones((B, S, S), dtype=np.int32),
        "Wq": rng.standard_normal((HID, HID), dtype=np.float32) * 0.02,
        "bq": rng.standard_normal((HID,), dtype=np.float32) * 0.01,
        "Wk": rng.standard_normal((HID, HID), dtype=np.float32) * 0.02,
        "bk": rng.standard_normal((HID,), dtype=np.float32) * 0.01,
        "Wv": rng.standard_normal((HID, HID), dtype=np.float32) * 0.02,
        "bv": rng.standard_normal((HID,), dtype=np.float32) * 0.01,
    }
    got = kernel(**ins)
    want = _numpy_fallback(**ins)
    err = np.abs(got - want).max() / np.abs(want).max()
    print("self-test rel err:", err)
